# revision 1
# baseline (speedup 1.0000x reference)
"""CSPN 3x3 propagation step on 8 trn2 NeuronCores (batch-parallel).

out[b, y, x] = sum_{t=(a,c)} guide[b, t, y+1, x+1] * src_t[...]
  src_t = hn shifted by (1-a, 1-c), center tap (a=c=1) uses h0.

Per core (B=1): row-shift accumulation is done on the TensorEngine with
exact 0/1 shift matrices (fp32 matmul is bit-exact for row selection),
column shifts are free-dim offsets on the DVE product ops.
"""

import numpy as np

import concourse.bacc as bacc
import concourse.mybir as mybir
from concourse import tile
from concourse.bass_utils import run_bass_kernel_spmd

F32 = mybir.dt.float32

B, H, W = 8, 352, 1216
HP, WP = H + 2, W + 2          # padded plane dims (354, 1218)
N_CORES = 8
# output row chunks: i0 = first padded out row, R = rows in chunk
CHUNKS = [(1, 126), (127, 126), (253, 100)]
# column strips (out padded col j0, width N)
STRIPS = [(1, 512), (513, 512), (1025, 192)]


def make_shift_mats():
    """S_d[k, m] = 1 iff k == m + d, for d in {0,1,2}; packed [128, 378]."""
    sm = np.zeros((128, 3 * 126), np.float32)
    for d in range(3):
        for m in range(126):
            sm[m + d, d * 126 + m] = 1.0
    return sm


def build(n_iters: int = 1):
    nc = bacc.Bacc()
    g_d = nc.dram_tensor("guide", [9, HP, WP], F32, kind="ExternalInput")
    hn_d = nc.dram_tensor("hn", [H, W], F32, kind="ExternalInput")
    h0_d = nc.dram_tensor("h0", [H, W], F32, kind="ExternalInput")
    sm_d = nc.dram_tensor("smat", [128, 3 * 126], F32, kind="ExternalInput")
    out_d = nc.dram_tensor("out", [H, W], F32, kind="ExternalOutput")

    with tile.TileContext(nc) as tc:
        with tc.tile_pool(name="const", bufs=1) as cpool, \
             tc.tile_pool(name="gpool", bufs=2) as gpool, \
             tc.tile_pool(name="spool", bufs=2) as spool, \
             tc.tile_pool(name="ppool", bufs=4) as ppool, \
             tc.tile_pool(name="opool", bufs=2) as opool, \
             tc.tile_pool(name="psum", bufs=2, space="PSUM") as pspool:

            smt = cpool.tile([128, 3 * 126], F32)
            nc.sync.dma_start(out=smt[:], in_=sm_d[:])

            # Split every load into two partition-halves, one per HWDGE
            # ring (SP + ACT sequencers), so both descriptor streams run
            # concurrently on every transfer. The GPSIMD SWDGE ring
            # measured ~2x slower — do not use it.
            def dual_dma(dst, src):
                n = dst.shape[0]
                m = n // 2
                if m == 0:
                    nc.sync.dma_start(out=dst, in_=src)
                    return
                nc.sync.dma_start(out=dst[0:m], in_=src[0:m])
                nc.scalar.dma_start(out=dst[m:n], in_=src[m:n])

            def chunk_body(i0, R):
                u0 = i0 - 1  # tile partition p <-> padded row u0 + p

                # ---- guide tiles: tap t partition k holds g_t row u0+k-Di
                # Out-of-plane partitions are zeroed (never selected by the
                # shift matrices; zeroing keeps PSUM NaN-free and CoreSim
                # race-clean).
                gts = []
                for t in range(9):
                    a = t // 3
                    di = 1 - a
                    gt = gpool.tile([128, WP], F32, tag=f"g{t}")
                    lo = u0 - di
                    lo_c, hi_c = max(lo, 0), min(lo + 128, HP)
                    d0 = lo_c - lo
                    if d0 > 0:
                        nc.vector.memset(gt[0:1, :], 0.0)
                    if d0 + (hi_c - lo_c) < 128:
                        nc.vector.memset(gt[96:128, :], 0.0)
                    dual_dma(gt[d0:d0 + hi_c - lo_c, :], g_d[t, lo_c:hi_c, :])
                    gts.append(gt)

                # ---- hn tile (padded plane window rows u0..u0+127)
                hnt = spool.tile([128, WP], F32, tag="hn")
                p_lo = max(0, 1 - u0)            # first partition with a real hn row
                p_hi = min(128, H + 1 - u0)      # one past last real row
                if p_hi < 128:
                    # bottom edge: zero the tail (covers pad row u=353) first
                    nc.vector.memset(hnt[96:128, :], 0.0)
                nc.vector.memset(hnt[:, 0:1], 0.0)
                nc.vector.memset(hnt[:, WP - 1:WP], 0.0)
                if p_lo > 0:
                    nc.vector.memset(hnt[0:1, :], 0.0)
                dual_dma(hnt[p_lo:p_hi, 1:WP - 1],
                         hn_d[u0 + p_lo - 1:u0 + p_hi - 1, :])

                # ---- h0 tile (same window; pads never selected)
                h0t = spool.tile([128, WP], F32, tag="h0")
                if p_lo > 0:
                    nc.vector.memset(h0t[0:1, :], 0.0)
                if p_hi < 128:
                    nc.vector.memset(h0t[96:128, :], 0.0)
                nc.vector.memset(h0t[:, 0:1], 0.0)
                nc.vector.memset(h0t[:, WP - 1:WP], 0.0)
                dual_dma(h0t[p_lo:p_hi, 1:WP - 1],
                         h0_d[u0 + p_lo - 1:u0 + p_hi - 1, :])

                # ---- products + shift-matmul accumulation
                psts = [pspool.tile([126, 512], F32, tag=f"ps{s}", name=f"ps{s}")
                        for s in range(len(STRIPS))]
                for t in range(9):
                    a, c = t // 3, t % 3
                    di = 1 - a
                    src = h0t if t == 4 else hnt
                    pt = ppool.tile([128, W], F32, tag="prod")
                    # P_t[k, w] = g_t[k, 1+w] * src[k, (2-c)+w]
                    cs = 1 if t == 4 else (2 - c)
                    nc.vector.tensor_tensor(pt[:, 0:W], gts[t][:, 1:1 + W],
                                            src[:, cs:cs + W],
                                            mybir.AluOpType.mult)
                    for s, (j0, N) in enumerate(STRIPS):
                        nc.tensor.matmul(psts[s][:, 0:N],
                                         smt[:, (di + 1) * 126:(di + 1) * 126 + 126],
                                         pt[:, j0 - 1:j0 - 1 + N],
                                         start=(t == 0), stop=(t == 8))

                # ---- PSUM -> SBUF -> HBM
                ot = opool.tile([126, W], F32, tag="out")
                for s, (j0, N) in enumerate(STRIPS):
                    nc.scalar.copy(out=ot[0:R, j0 - 1:j0 - 1 + N],
                                   in_=psts[s][0:R, 0:N])
                dual_dma(out_d[i0 - 1:i0 - 1 + R, :], ot[0:R, 0:W])

            def body(_iv=None):
                for i0, R in CHUNKS:
                    chunk_body(i0, R)

            if n_iters == 1:
                body()
            else:
                with tc.For_i(0, n_iters, 1):
                    body()

    nc.finalize()
    return nc


_nc_cache = {}


def _get_nc(n_iters=1):
    if n_iters not in _nc_cache:
        _nc_cache[n_iters] = build(n_iters)
    return _nc_cache[n_iters]


def kernel(guide_weight: np.ndarray, hn: np.ndarray, h0: np.ndarray) -> np.ndarray:
    """Full inputs: guide_weight [8,9,354,1218], hn/h0 [8,1,352,1216] f32.
    Returns [8,1,352,1216] f32."""
    nc = _get_nc(1)
    sm = make_shift_mats()
    in_maps = [
        {
            "guide": np.ascontiguousarray(guide_weight[b], dtype=np.float32),
            "hn": np.ascontiguousarray(hn[b, 0], dtype=np.float32),
            "h0": np.ascontiguousarray(h0[b, 0], dtype=np.float32),
            "smat": sm,
        }
        for b in range(B)
    ]
    res = run_bass_kernel_spmd(nc, in_maps, list(range(N_CORES)))
    out = np.stack([res.results[b]["out"] for b in range(B)], axis=0)
    return out[:, None].astype(np.float32)



# revision 2
# speedup vs baseline: 129.1705x; 129.1705x over previous
"""CSPN 3x3 propagation on 8 trn2 NeuronCores (batch-parallel).

out[y, x] = sum_{a,c} g[3a+c, y+1, x+1] * src[y+1-a, x+1-c]
  (src = hn zero-padded; the center tap a=c=1 uses h0)

Per core (B=1), row chunks of <=126 output rows. Guide tap planes are
DMA-loaded row-shifted by di=1-a so the DVE product
pt[k] = g_t[u0+k-di] * hn_padded[u0+k] is partition-aligned; the
TensorEngine row-shifts and sums the 9 products in PSUM via exact 0/1
shift matrices (bf16: 1 cycle/row; fp32 would be 4x).

DMA shapes: HWDGE only fans a transfer across the 16 SDMA engine slots
for certain partition counts (measured: 64/96/112/120 spread evenly;
125..128 collapse onto slot 0 at ~1/4 bandwidth). All transfers here are
exact [0:64]/[64:128] halves, one per HWDGE ring. Inputs are host-padded
so every load is a full 128-row window:
  guide  -> flat [3216, 1218]: row 0 zero, then the 9 planes, zero tail
  hn/h0  -> [384, 1216]: row 0 zero, rows 1..352 data, zero tail
  out    -> [384, 1216]: chunk ci stores its 128-row window at ci*128
            (disjoint slots; host reassembles the valid R rows of each)
"""

import numpy as np
import ml_dtypes

import concourse.bacc as bacc
import concourse.mybir as mybir
from concourse import tile
from concourse.bass_utils import run_bass_kernel_spmd

F32 = mybir.dt.float32
BF16 = mybir.dt.bfloat16

B, H, W = 8, 352, 1216
HP, WP = H + 2, W + 2          # padded plane dims (354, 1218)
N_CORES = 8
GROWS = 3216                   # padded flat guide rows (1 + 9*354 + 29)
SROWS = 384                    # padded hn/h0/out rows
CHUNKS = [(1, 126), (127, 126), (253, 100)]   # (i0 = first padded out row, R)
STRIPS = [(0, 512), (512, 512), (1024, 192)]  # (out col w0, width N)


def make_shift_mats():
    """S_d[k, m] = 1 iff k == m + d, d in {0,1,2}; packed [128, 378] bf16."""
    sm = np.zeros((128, 3 * 126), ml_dtypes.bfloat16)
    for d in range(3):
        for m in range(126):
            sm[m + d, d * 126 + m] = 1.0
    return sm


def prep_core_inputs(guide_b: np.ndarray, hn_b: np.ndarray, h0_b: np.ndarray,
                     sm: np.ndarray) -> dict:
    """Pad one sample's inputs to the kernel's DMA-friendly layouts.
    guide_b [9, 354, 1218], hn_b/h0_b [352, 1216] -> dram input dict."""
    gp = np.zeros((GROWS, WP), np.float32)
    gp[1:1 + 9 * HP] = np.asarray(guide_b, np.float32).reshape(9 * HP, WP)
    hp = np.zeros((SROWS, W), np.float32)
    hp[1:1 + H] = np.asarray(hn_b, np.float32)
    h0p = np.zeros((SROWS, W), np.float32)
    h0p[1:1 + H] = np.asarray(h0_b, np.float32)
    return {"guide": gp, "hn": hp, "h0": h0p, "smat": sm}


def assemble_out(out_pad: np.ndarray) -> np.ndarray:
    """[384, 1216] chunk slots -> [352, 1216]."""
    parts = [out_pad[ci * 128:ci * 128 + R] for ci, (_, R) in enumerate(CHUNKS)]
    return np.concatenate(parts, axis=0)


def build(n_iters: int = 1):
    nc = bacc.Bacc()
    g_d = nc.dram_tensor("guide", [GROWS, WP], F32, kind="ExternalInput")
    hn_d = nc.dram_tensor("hn", [SROWS, W], F32, kind="ExternalInput")
    h0_d = nc.dram_tensor("h0", [SROWS, W], F32, kind="ExternalInput")
    sm_d = nc.dram_tensor("smat", [128, 3 * 126], BF16, kind="ExternalInput")
    out_d = nc.dram_tensor("out", [SROWS, W], F32, kind="ExternalOutput")

    with tile.TileContext(nc) as tc:
        with tc.tile_pool(name="const", bufs=1) as cpool, \
             tc.tile_pool(name="gpool", bufs=3) as gpool, \
             tc.tile_pool(name="spool", bufs=2) as spool, \
             tc.tile_pool(name="ppool", bufs=4) as ppool, \
             tc.tile_pool(name="opool", bufs=2) as opool, \
             tc.tile_pool(name="psum", bufs=2, space="PSUM") as pspool:

            smt = cpool.tile([128, 3 * 126], BF16)
            nc.sync.dma_start(out=smt[:], in_=sm_d[:])

            engs = [nc.sync, nc.scalar]

            def dual64(dst, src, flip=0):
                """dst[0:128] <- src, exact 64-row halves, one per ring."""
                engs[flip].dma_start(out=dst[0:64], in_=src[0:64])
                engs[1 - flip].dma_start(out=dst[64:128], in_=src[64:128])

            def chunk_body(ci, i0, R):
                u0 = i0 - 1          # hn tile partition k <-> padded row u0+k
                KL = 128 if R > 100 else 112   # rows per load (112 spreads too)

                # ---- hn/h0 first: every product reads them, so they must
                # lead the per-ring FIFO ahead of the guide stream.
                hnt = spool.tile([128, WP], F32, tag="hn")
                nc.vector.memset(hnt[:, 0:1], 0.0)
                nc.vector.memset(hnt[:, WP - 1:WP], 0.0)
                if KL == 128:
                    engs[0].dma_start(out=hnt[0:64, 1:WP - 1],
                                      in_=hn_d[u0:u0 + 64, :])
                    engs[1].dma_start(out=hnt[64:128, 1:WP - 1],
                                      in_=hn_d[u0 + 64:u0 + 128, :])
                else:
                    engs[0].dma_start(out=hnt[0:KL, 1:WP - 1],
                                      in_=hn_d[u0:u0 + KL, :])

                h0t = spool.tile([128, W], F32, tag="h0")
                if KL == 128:
                    dual64(h0t, h0_d[u0:u0 + 128, :], flip=1)
                else:
                    engs[1].dma_start(out=h0t[0:KL, :], in_=h0_d[u0:u0 + KL, :])

                # ---- guide tiles: partition k of tile t holds g_t row u0+k-di
                gts = []
                for t in range(9):
                    a = t // 3
                    di = 1 - a
                    gt = gpool.tile([128, WP], F32, tag=f"g{t}")
                    base = 1 + t * HP + u0 - di
                    if KL == 128:
                        dual64(gt, g_d[base:base + 128, :], flip=t % 2)
                    else:
                        engs[t % 2].dma_start(out=gt[0:KL, :],
                                              in_=g_d[base:base + KL, :])
                    gts.append(gt)

                # ---- products (bf16) + shift-matmul accumulation
                psts = [pspool.tile([126, 512], F32, tag=f"ps{s}", name=f"ps{s}")
                        for s in range(len(STRIPS))]
                for t in range(9):
                    a, c = t // 3, t % 3
                    d = 2 - a
                    pt = ppool.tile([128, W], BF16, tag="prod")
                    if t == 4:
                        nc.vector.tensor_tensor(pt[0:KL, 0:W],
                                                gts[t][0:KL, 1:1 + W],
                                                h0t[0:KL, 0:W],
                                                mybir.AluOpType.mult)
                    else:
                        nc.vector.tensor_tensor(pt[0:KL, 0:W],
                                                gts[t][0:KL, 1:1 + W],
                                                hnt[0:KL, 2 - c:2 - c + W],
                                                mybir.AluOpType.mult)
                    for s, (w0, N) in enumerate(STRIPS):
                        nc.tensor.matmul(psts[s][0:R, 0:N],
                                         smt[0:KL, d * 126:d * 126 + R],
                                         pt[0:KL, w0:w0 + N],
                                         start=(t == 0), stop=(t == 8))

                # ---- PSUM -> SBUF -> HBM (disjoint 128-row slot per chunk)
                ot = opool.tile([128, W], F32, tag="out")
                nc.gpsimd.memset(ot[96:KL, :], 0.0)
                for s, (w0, N) in enumerate(STRIPS):
                    nc.scalar.copy(out=ot[0:R, w0:w0 + N], in_=psts[s][0:R, 0:N])
                if KL == 128:
                    dual64(out_d[ci * 128:ci * 128 + 128, :], ot, flip=ci % 2)
                else:
                    engs[ci % 2].dma_start(out=out_d[ci * 128:ci * 128 + KL, :],
                                           in_=ot[0:KL, :])

            def body(_iv=None):
                for ci, (i0, R) in enumerate(CHUNKS):
                    chunk_body(ci, i0, R)

            if n_iters == 1:
                body()
            else:
                with tc.For_i(0, n_iters, 1):
                    body()

    nc.finalize()
    return nc


_nc_cache = {}


def _get_nc(n_iters=1):
    if n_iters not in _nc_cache:
        _nc_cache[n_iters] = build(n_iters)
    return _nc_cache[n_iters]


def kernel(guide_weight: np.ndarray, hn: np.ndarray, h0: np.ndarray) -> np.ndarray:
    """Full inputs: guide_weight [8,9,354,1218], hn/h0 [8,1,352,1216] f32.
    Returns [8,1,352,1216] f32."""
    nc = _get_nc(1)
    sm = make_shift_mats()
    in_maps = [prep_core_inputs(guide_weight[b], hn[b, 0], h0[b, 0], sm)
               for b in range(B)]
    res = run_bass_kernel_spmd(nc, in_maps, list(range(N_CORES)))
    out = np.stack([assemble_out(res.results[b]["out"]) for b in range(B)], axis=0)
    return out[:, None].astype(np.float32)
